# revision 17
# baseline (speedup 1.0000x reference)
"""Multi-head attention (cosine-similarity scores, q=k=v) on 8 trn2 cores.

Reference computation (per head h, batch b):
    h_bh = sin_b @ Wx_h + bx_h                       # [S, F]
    C    = (h_bh h_bh^T) / (|h_s||h_t|)              # cosine scores, symmetric
    P    = softmax(C, axis=-1)                       # no max-shift needed: |C|<=1
    out_bh = P @ h_bh                                # [S, F]
    out_b  = concat_h(out_bh) @ Wp + bp              # [S, D]

Sharding: tensor-parallel over heads. Each core owns HPC=2 heads, computes the
partial output projection for its heads over the full batch, and the host sums
the 8 partials (+bp).

Layout: all score/value matmuls run column-wise [t-partition, s-free]; C's
symmetry makes exp(C) stored column-wise exactly the E[s,t] operand for
Y^T = h^T E.  The two heads live on partitions 0-63 / 64-127, so their K=64
score matmuls run CONCURRENTLY in the PE array (row-group tiling) and one
[128,1024] exp covers both heads (ACT is the critical engine; its per-call
overhead is amortized at the max width 4 PSUM banks allow).  The softmax
denominator rides as a ones-column in the Y stationary (psum row 64).  rsqrt
of the norms is exp(-0.5*ln(x)) so the whole kernel uses ONE ACT table set
(natural_log_exp_and_others) - no table swaps.  Norm reciprocals are
partition-broadcast with tiny K=1 matmuls (gpsimd broadcast can only read
partition 0; PE does it for free in the slack).
"""
import numpy as np

import concourse.bacc as bacc
import concourse.tile as tile
import concourse.mybir as mybir
from concourse import bass_isa, masks
from concourse.bass_utils import run_bass_kernel_spmd

B, S, D, H, F = 4, 2048, 1024, 16, 64
NCORES = 8
HPC = H // NCORES          # 2 heads per core
FL = HPC * F               # 128 local feature columns
SCH = 512                  # s-chunk width (one psy bank)
NCH = S // SCH             # 4 chunks per batch
KT = D // 128              # 8 contraction tiles for the input projection
NT0 = S // 128             # 16 t-blocks
QH = 1024                  # sin DMA block width
AUGW = NT0 * (F + 1)       # 1040 aug columns per head

FP = mybir.dt.float32
BF = mybir.dt.bfloat16
NP_BF = mybir.dt.np(mybir.dt.bfloat16)
AF = mybir.ActivationFunctionType


def _patch_act_tables():
    """Force Ln and Exp to resolve to the combined
    natural_log_exp_and_others set: with the default tables the chooser
    puts them in different sets and reloads ACT tables between every
    ln/exp pair (1.3us each, 16 loads per kernel).  Emptying the two
    narrower sets (order and ids preserved) leaves the combined set as
    the only candidate, so the whole kernel runs on ONE table load."""
    import concourse.hw_specs as hw_specs

    if getattr(bacc, "_act_tables_patched", False):
        return
    orig = hw_specs.get_activation_tables

    def patched(arch):
        t = dict(orig(arch))
        for k in ("exp_and_others", "natural_log"):
            if k in t:
                t[k] = set()
        return t

    bacc.get_activation_tables = patched
    bacc._act_tables_patched = True


def _build_nc():
    _patch_act_tables()
    nc = bacc.Bacc("TRN2", target_bir_lowering=False, debug=False)

    sinT = nc.dram_tensor("sinT", [B, KT, 128, S], BF,
                          kind="ExternalInput")
    wxl = nc.dram_tensor("wxl", [128, KT * FL], BF, kind="ExternalInput")
    bxl = nc.dram_tensor("bxl", [FL, 1], FP, kind="ExternalInput")
    wpl = nc.dram_tensor("wpl", [FL, D], BF, kind="ExternalInput")
    outp = nc.dram_tensor("outp", [B, S, D], BF, kind="ExternalOutput")

    with tile.TileContext(nc) as tc:
        with (
            tc.tile_pool(name="const", bufs=1) as constp,
            tc.tile_pool(name="wpool", bufs=1) as wpool,
            tc.tile_pool(name="sin", bufs=8) as sinp,
            tc.tile_pool(name="pa", bufs=1) as pa,
            tc.tile_pool(name="pb", bufs=2) as pb,
            tc.tile_pool(name="epool", bufs=4) as epool,
            tc.tile_pool(name="tailp", bufs=2) as tailp,
            tc.tile_pool(name="opool", bufs=3) as opool,
            # 8 PSUM banks:
            #   ps_c  2 x [128,1024] = 4 banks (paired score blocks)
            #   ps_y  2 x [65,512]   = 2 banks (per-head Y accumulators)
            #   ps_a  1 x [128,512]  = 1 bank  (proj accum / psn1 / rnb / outproj)
            #   ps_s  1 x [128,512]  = 1 bank  (psn0 / rnb / transposes / outproj)
            tc.tile_pool(name="ps_c", bufs=2, space="PSUM") as ps_c,
            tc.tile_pool(name="ps_y", bufs=1, space="PSUM") as ps_y,
            tc.tile_pool(name="ps_a", bufs=1, space="PSUM") as ps_a,
            tc.tile_pool(name="ps_s", bufs=1, space="PSUM") as ps_s,
        ):
            # ---- constants / weights ----
            ident = constp.tile([128, 128], FP, tag="ident")
            masks.make_identity(nc, ident[:])

            # ones2[:, h] = 1 on partitions h*64..h*64+63 (norm reductions)
            ones2_f = constp.tile([128, 2], FP, tag="ones2f")
            nc.vector.memset(ones2_f[:], 0.0)
            nc.vector.memset(ones2_f[0:64, 0:1], 1.0)
            nc.vector.memset(ones2_f[64:128, 1:2], 1.0)
            ones2 = constp.tile([128, 2], BF, tag="ones2")
            nc.vector.tensor_copy(ones2[:], ones2_f[:])

            # all-ones [128,64] bf16: K=1 broadcast-matmul stationary
            onesb_f = constp.tile([128, 64], FP, tag="onesbf")
            nc.vector.memset(onesb_f[:], 1.0)
            onesb = constp.tile([128, 64], BF, tag="onesb")
            nc.vector.tensor_copy(onesb[:], onesb_f[:])

            ones16_f = constp.tile([128, NT0], FP, tag="ones16f")
            nc.vector.memset(ones16_f[:], 1.0)

            wx_t = wpool.tile([128, KT * FL], BF, tag="wx")
            nc.sync.dma_start(wx_t[:], wxl.ap())
            bx_t = wpool.tile([FL, 1], FP, tag="bx")
            nc.sync.dma_start(bx_t[:], bxl.ap())
            wp_t = wpool.tile([FL, D], BF, tag="wp")
            nc.sync.dma_start(wp_t[:], wpl.ap())

            state = {}

            def a_parts(b):
                """Emitters for phase A of batch b: projection, norms,
                normalization, aug build.  Returned as a list of parts to be
                interleaved into the previous batch's attention chunks."""
                st = {}
                state[b] = st
                sints = {}

                def mk():
                    st["hT"] = pa.tile([128, S], FP, tag="hT", name=f"hT_{b}")
                    st["sqt"] = pa.tile([128, S], BF, tag="sq", name=f"sq_{b}")
                    st["hTn"] = pb.tile([128, S], BF, tag="hTn",
                                        name=f"hTn_{b}")
                    st["aug"] = pb.tile([128, HPC * AUGW], BF, tag="aug",
                                        name=f"aug_{b}")
                    st["outT"] = pb.tile([128, S], BF, tag="outT",
                                         name=f"outT_{b}")
                    st["ysb"] = [
                        pb.tile([F + 1, S], FP, tag=f"ysb{h}",
                                name=f"ysb{h}_{b}")
                        for h in range(HPC)
                    ]
                    st["rn"] = [
                        pa.tile([128, SCH], BF, tag=f"rn{h}",
                                name=f"rn{h}_{b}")
                        for h in range(HPC)
                    ]

                def a1d():
                    # one 512KB DMA per k-tile (sync-engine dispatch is
                    # ~650ns per DMA; fewer, bigger transfers).  Issued a
                    # full batch ahead so the proj matmuls never reach the
                    # in-order PE queue head while their data is in flight
                    # (head-of-line blocking there starves ACT).
                    for k in range(KT):
                        sint = sinp.tile([128, S], BF, tag="sin",
                                         name=f"sin_{b}_{k}")
                        nc.sync.dma_start(sint[:], sinT.ap()[b, k])
                        sints[k] = sint

                def a1q(half, q):
                    # one half (4 k-tiles) of a 512-wide projection accum
                    if half == 0 and q == 0:
                        mk()
                    cs = slice(half * SCH, (half + 1) * SCH)
                    if q == 0:
                        st["pshT"] = ps_a.tile([128, SCH], FP, tag="a",
                                               name=f"pshT_{b}_{half}")
                    pshT = st["pshT"]
                    for k in range(q * KT // 2, (q + 1) * KT // 2):
                        nc.tensor.matmul(
                            pshT[:], wx_t[:, k * FL:(k + 1) * FL],
                            sints[k][:, cs],
                            start=(k == 0), stop=(k == KT - 1),
                        )
                    if q == 1:
                        nc.vector.tensor_scalar_add(st["hT"][:, cs], pshT[:],
                                                    bx_t[:])
                        nc.vector.tensor_mul(st["sqt"][:, cs], st["hT"][:, cs],
                                             st["hT"][:, cs])

                def norms(h):
                    # norm^2 rows for head h -> psum partitions {0,32,64,96}
                    # (chunk c at row c*32), then rnorm = exp(-0.5*ln(x)):
                    # same ACT table set as the attention exp, no table swaps.
                    # Compact: the psum slot is created and consumed within
                    # this one part (the 1-buf pools rotate with outproj).
                    pool = ps_s if h == 0 else ps_a
                    psn = pool.tile([128, SCH], FP, tag="s" if h == 0
                                    else "a", name=f"psn{h}_{b}")
                    for c in range(NCH):
                        cs = slice(c * SCH, (c + 1) * SCH)
                        nc.tensor.matmul(psn[c * 32:c * 32 + 1, :],
                                         ones2[:, h:h + 1], st["sqt"][:, cs],
                                         start=True, stop=True,
                                         tile_position=(0, c * 32))
                    lnt = pa.tile([128, SCH], FP, tag="lnt",
                                  name=f"lnt_{b}_{h}")
                    nc.scalar.activation(lnt[:], psn[:], AF.Ln)
                    nc.scalar.activation(st["rn"][h][:], lnt[:], AF.Exp,
                                         scale=-0.5)

                def rnb(c):
                    # broadcast rnorm rows to [128,512] via K=1 matmuls,
                    # then one mul makes the normalized hTn chunk
                    pool = ps_s if c % 2 == 0 else ps_a
                    cs = slice(c * SCH, (c + 1) * SCH)
                    prn = pool.tile([128, SCH], FP, tag="s" if c % 2 == 0
                                    else "a", name=f"prn_{b}_{c}")
                    r = c * 32
                    nc.tensor.matmul(prn[0:64, :], onesb[r:r + 1, :],
                                     st["rn"][0][r:r + 1, :],
                                     start=True, stop=True,
                                     tile_position=(r, 0))
                    nc.tensor.matmul(prn[64:128, :], onesb[r:r + 1, :],
                                     st["rn"][1][r:r + 1, :],
                                     start=True, stop=True,
                                     tile_position=(r, 64))
                    nc.vector.tensor_mul(st["hTn"][:, cs], st["hT"][:, cs],
                                         prn[:])

                def a3(q):
                    # 4 transposes: hT [f,t] -> aug [t,f] for both heads
                    for t0 in range(q * 4, q * 4 + 4):
                        pool = ps_s if t0 % 2 == 0 else ps_a
                        pst = pool.tile([128, 128], FP, tag="s" if t0 % 2 == 0
                                        else "a", name=f"pst_{b}_{t0}")
                        nc.tensor.transpose(
                            pst[:], st["hT"][:, t0 * 128:(t0 + 1) * 128],
                            ident[:]
                        )
                        dst = st["aug"][:].rearrange(
                            "p (h t f) -> p h t f", h=HPC, f=F + 1
                        )[:, :, t0, 0:F]
                        src = pst[:].rearrange("p (h f) -> p h f", h=HPC)
                        nc.vector.tensor_copy(dst, src)

                def a4():
                    for h in range(HPC):
                        ones_col = st["aug"][:].rearrange(
                            "p (h t f) -> p h t f", h=HPC, f=F + 1
                        )[:, h, :, F:F + 1]
                        nc.vector.tensor_copy(ones_col, ones16_f[:])

                parts = (
                    [lambda: a1q(0, 0), lambda: a1q(0, 1),
                     lambda: a1q(1, 0), lambda: a1q(1, 1),
                     lambda: a1q(2, 0), lambda: a1q(2, 1),
                     lambda: a1q(3, 0), lambda: a1q(3, 1),
                     lambda: norms(0), lambda: norms(1)]
                    + [lambda c=c: rnb(c) for c in range(NCH)]
                    + [lambda q=q: a3(q) for q in range(4)]
                    + [a4]
                )
                return a1d, parts

            def b_chunk(b, c, fillers=(), end_fillers=()):
                """One 512-wide s-chunk: both heads, all 16 t-blocks."""
                st = state[b]
                cs = slice(c * SCH, (c + 1) * SCH)
                fillers = list(fillers)
                psy = [
                    ps_y.tile([F + 1, SCH], FP, tag=f"y{h}",
                              name=f"psy{h}_{b}_{c}")
                    for h in range(HPC)
                ]
                aug4 = st["aug"][:].rearrange("p (h t f) -> p h t f",
                                              h=HPC, f=F + 1)
                for t0 in range(NT0):
                    ts0 = slice(t0 * 128, (t0 + 1) * 128)
                    psc = ps_c.tile([128, 2 * SCH], FP, tag="c",
                                    name=f"psc_{b}_{c}_{t0}")
                    for h in range(HPC):
                        hs = slice(h * F, (h + 1) * F)
                        nc.tensor.matmul(
                            psc[:, h * SCH:(h + 1) * SCH],
                            st["hTn"][hs, ts0], st["hTn"][hs, cs],
                            start=True, stop=True,
                        )
                    et = epool.tile([128, 2 * SCH], BF, tag="E",
                                    name=f"E_{b}_{c}_{t0}")
                    nc.scalar.activation(et[:], psc[:], AF.Exp)
                    for h in range(HPC):
                        nc.tensor.matmul(
                            psy[h][:], aug4[:, h, t0, :],
                            et[:, h * SCH:(h + 1) * SCH],
                            start=(t0 == 0), stop=(t0 == NT0 - 1),
                        )
                    if fillers:
                        fillers.pop(0)()
                # release the psy banks quickly into the per-head accums
                for h in range(HPC):
                    nc.vector.tensor_copy(st["ysb"][h][:, cs], psy[h][:])
                for part in fillers:
                    part()
                for part in end_fillers:
                    part()

            def tail(b, h, half):
                """Divide accumulated Y by the softmax denominators."""
                st = state[b]
                hs2 = slice(half * QH, (half + 1) * QH)
                rdsrc = tailp.tile([1, QH], FP, tag="rdsrc",
                                   name=f"rdsrc_{b}_{h}_{half}")
                nc.vector.tensor_copy(rdsrc[:], st["ysb"][h][F:F + 1, hs2])
                rd = tailp.tile([1, QH], FP, tag="rd",
                                name=f"rd_{b}_{h}_{half}")
                nc.vector.reciprocal_approx_fast(rd[:], rdsrc[:])
                rdb = tailp.tile([F, QH], FP, tag="rdb",
                                 name=f"rdb_{b}_{h}_{half}")
                nc.gpsimd.partition_broadcast(rdb[:], rd[:])
                nc.vector.tensor_mul(st["outT"][h * F:(h + 1) * F, hs2],
                                     st["ysb"][h][0:F, hs2], rdb[:])

            def c_parts(b):
                st = state[b]

                def c1(sb, scalar_copy=False):
                    ss = slice(sb * 128, (sb + 1) * 128)
                    ot = opool.tile([128, D], BF, tag="osb",
                                    name=f"ot_{b}_{sb}")
                    for n in range(D // 512):
                        pool = ps_a if n == 0 else ps_s
                        psp = pool.tile([128, 512], FP, tag="a" if n == 0
                                        else "s", name=f"psp_{b}_{sb}_{n}")
                        nc.tensor.matmul(
                            psp[:], st["outT"][:, ss],
                            wp_t[:, n * 512:(n + 1) * 512],
                            start=True, stop=True,
                        )
                        if scalar_copy and n == 0:
                            nc.scalar.copy(ot[:, n * 512:(n + 1) * 512],
                                           psp[:])
                        else:
                            nc.vector.tensor_copy(
                                ot[:, n * 512:(n + 1) * 512], psp[:])
                    nc.sync.dma_start(outp.ap()[b, ss, :], ot[:])

                return [
                    lambda sb=sb: c1(sb, scalar_copy=(b == B - 1 and sb >= 8))
                    for sb in range(S // 128)
                ]

            # ---- software-pipelined emission ----
            # Chunks of batch b interleave with phase A of b+1 and the
            # output projections whose outT halves are complete.  Each
            # batch's sin DMAs are issued one full batch early (chunk 3 of
            # b-1 runs a1d of b+1) so proj matmuls never wait on DMA.
            cl = {}
            dma0, parts0 = a_parts(0)
            dma0()
            for part in parts0:
                part()
            nxt = a_parts(1) if B > 1 else None
            if nxt:
                nxt[0]()
            for b in range(B):
                cl[b] = c_parts(b)
                ap = nxt[1] if nxt else []
                nxt2 = a_parts(b + 2) if b + 2 < B else None
                cprev = cl[b - 1][8:16] if b >= 1 else []
                ccur = cl[b][0:8]
                plan = [
                    (0, cprev[0:4] + ap[0:5], []),
                    (1, cprev[4:8] + ap[5:10],
                     [lambda: tail(b, 0, 0)]),
                    (2, [lambda: tail(b, 1, 0)] + ap[10:19], []),
                    (3, ccur[0:8] + ([nxt2[0]] if nxt2 else []),
                     [lambda: tail(b, 0, 1), lambda: tail(b, 1, 1)]),
                ]
                for c, fillers, endf in plan:
                    b_chunk(b, c, fillers, endf)
                nxt = nxt2
            for part in cl[B - 1][8:16]:
                part()

    nc.compile()
    return nc

_NC_CACHE = []


def _get_nc():
    if not _NC_CACHE:
        _NC_CACHE.append(_build_nc())
    return _NC_CACHE[0]


def make_in_maps(sin, Wx, bx, Wp):
    """Host-side sharding: per-core input dicts."""
    # [B, D, S] -> contiguous slabs [B, KT, 128, S]: each sin DMA is one
    # contiguous 512KB read
    sinT = np.ascontiguousarray(
        np.transpose(sin, (0, 2, 1)).reshape(B, KT, 128, S).astype(NP_BF)
    )
    in_maps = []
    for c in range(NCORES):
        hs = slice(c * HPC, (c + 1) * HPC)
        # [D, FL] stacked head projections -> [128, KT*FL] k-tile-major
        wxl = np.concatenate([Wx[h] for h in range(c * HPC, (c + 1) * HPC)],
                             axis=1)
        wxl = np.ascontiguousarray(
            wxl.reshape(KT, 128, FL).transpose(1, 0, 2).reshape(128, KT * FL)
        ).astype(NP_BF)
        bxl = np.ascontiguousarray(bx[hs].reshape(FL, 1))
        wpl = np.ascontiguousarray(Wp[c * FL:(c + 1) * FL, :]).astype(NP_BF)
        in_maps.append({"sinT": sinT, "wxl": wxl, "bxl": bxl, "wpl": wpl})
    return in_maps


def make_runner(sin, Wx, bx, Wp):
    """Build a repeat-callable single-execution runner with device-resident
    inputs.

    Outputs are fed back as the donated output buffers, so each call is
    dispatch + device execution only (no host transfers). Returns
    (run_once, block) where run_once() dispatches one execution
    asynchronously and block() waits for all dispatched work.
    """
    import jax
    from concourse import bass2jax as b2j
    from concourse import mybir as _mb

    nc = _get_nc()
    b2j.install_neuronx_cc_hook()
    in_maps = make_in_maps(
        np.asarray(sin, np.float32), np.asarray(Wx, np.float32),
        np.asarray(bx, np.float32), np.asarray(Wp, np.float32),
    )

    in_names, out_names, out_avals, zero_outs = [], [], [], []
    for alloc in nc.m.functions[0].allocations:
        if not isinstance(alloc, _mb.MemoryLocationSet):
            continue
        name = alloc.memorylocations[0].name
        if alloc.kind == "ExternalInput":
            if nc.partition_id_tensor is None or name != nc.partition_id_tensor.name:
                in_names.append(name)
        elif alloc.kind == "ExternalOutput":
            out_names.append(name)
            shape = tuple(alloc.tensor_shape)
            dtype = _mb.dt.np(alloc.dtype)
            out_avals.append(jax.core.ShapedArray(shape, dtype))
            zero_outs.append(np.zeros(shape, dtype))
    n_params = len(in_names)
    n_outs = len(out_avals)
    all_names = in_names + out_names
    donate = tuple(range(n_params, n_params + n_outs))

    pid_name = nc.partition_id_tensor.name if nc.partition_id_tensor else None
    body_names = all_names + ([pid_name] if pid_name else [])

    def _exec_once(ins_, outs_):
        operands = list(ins_) + list(outs_)
        if pid_name:
            operands.append(b2j.partition_id_tensor())
        outs = b2j._bass_exec_p.bind(
            *operands,
            out_avals=tuple(out_avals),
            in_names=tuple(body_names),
            out_names=tuple(out_names),
            lowering_input_output_aliases=(),
            sim_require_finite=True,
            sim_require_nnan=True,
            nc=nc,
        )
        return tuple(outs)

    def _body(*args):
        return _exec_once(args[:n_params], args[n_params:])

    devices = jax.devices()[:NCORES]
    mesh = b2j.Mesh(np.asarray(devices), ("core",))
    in_specs = (b2j.PartitionSpec("core"),) * (n_params + n_outs)
    out_specs = (b2j.PartitionSpec("core"),) * n_outs
    sharded = jax.jit(
        b2j.shard_map(_body, mesh=mesh, in_specs=in_specs,
                      out_specs=out_specs, check_rep=False),
        donate_argnums=donate, keep_unused=True,
    )
    sharding = jax.sharding.NamedSharding(mesh, b2j.PartitionSpec("core"))
    concat_in = [
        jax.device_put(
            np.concatenate([np.asarray(in_maps[c][nm]) for c in range(NCORES)],
                           axis=0),
            sharding,
        )
        for nm in in_names
    ]
    outs = [
        jax.device_put(np.zeros((NCORES * z.shape[0], *z.shape[1:]), z.dtype),
                       sharding)
        for z in zero_outs
    ]
    jax.block_until_ready(concat_in)

    state = {"outs": outs}

    def run_once():
        state["outs"] = sharded(*concat_in, *state["outs"])

    def block():
        jax.block_until_ready(state["outs"])

    return run_once, block


def benchmark(sin, Wx, bx, Wp, iters=10, loop_n=1, runner=None):
    """Timed loop of the compiled executable; returns per-exec ns."""
    import time as _time

    run_once, block = runner or make_runner(sin, Wx, bx, Wp)
    times = []
    for _ in range(iters):
        t0 = _time.perf_counter()
        for _k in range(loop_n):
            run_once()
        block()
        times.append((_time.perf_counter() - t0) * 1e9 / loop_n)
    return times


def kernel(sin, Wx, bx, Wp, bp, _trace=False):
    sin = np.asarray(sin, dtype=np.float32)
    Wx = np.asarray(Wx, dtype=np.float32)
    bx = np.asarray(bx, dtype=np.float32)
    Wp = np.asarray(Wp, dtype=np.float32)
    bp = np.asarray(bp, dtype=np.float32)

    nc = _get_nc()
    in_maps = make_in_maps(sin, Wx, bx, Wp)
    res = run_bass_kernel_spmd(nc, in_maps, list(range(NCORES)), trace=_trace)
    out = np.sum(np.stack([np.asarray(r["outp"], np.float32)
                       for r in res.results]), axis=0) + bp
    if _trace:
        kernel.last_results = res
    return out.astype(np.float32)


# revision 21
# speedup vs baseline: 1.1259x; 1.1259x over previous
"""Multi-head attention (cosine-similarity scores, q=k=v) on 8 trn2 cores.

Reference computation (per head h, batch b):
    h_bh = sin_b @ Wx_h + bx_h                       # [S, F]
    C    = (h_bh h_bh^T) / (|h_s||h_t|)              # cosine scores, symmetric
    P    = softmax(C, axis=-1)                       # no max-shift needed: |C|<=1
    out_bh = P @ h_bh                                # [S, F]
    out_b  = concat_h(out_bh) @ Wp + bp              # [S, D]

Sharding: tensor-parallel over heads. Each core owns HPC=2 heads, computes the
partial output projection for its heads over the full batch, and the host sums
the 8 partials (+bp).

Layout: all score/value matmuls run column-wise [t-partition, s-free]; C's
symmetry makes exp(C) stored column-wise exactly the E[s,t] operand for
Y^T = h^T E.  The two heads live on partitions 0-63 / 64-127, so their K=64
score matmuls run CONCURRENTLY in the PE array (row-group tiling) and one
[128,1024] exp covers both heads (ACT is the critical engine; its per-call
overhead is amortized at the max width 4 PSUM banks allow).  The softmax
denominator rides as a ones-column in the Y stationary (psum row 64).  rsqrt
of the norms is exp(-0.5*ln(x)) so the whole kernel uses ONE ACT table set
(natural_log_exp_and_others) - no table swaps.  Norm reciprocals are
partition-broadcast with tiny K=1 matmuls (gpsimd broadcast can only read
partition 0; PE does it for free in the slack).
"""
import numpy as np

import concourse.bacc as bacc
import concourse.tile as tile
import concourse.mybir as mybir
from concourse import bass_isa, masks
from concourse.bass_utils import run_bass_kernel_spmd

B, S, D, H, F = 4, 2048, 1024, 16, 64
NCORES = 8
HPC = H // NCORES          # 2 heads per core
FL = HPC * F               # 128 local feature columns
SCH = 512                  # s-chunk width (one psy bank)
NCH = S // SCH             # 4 chunks per batch
KT = D // 128              # 8 contraction tiles for the input projection
NT0 = S // 128             # 16 t-blocks
QH = 1024                  # sin DMA block width
AUGW = NT0 * (F + 1)       # 1040 aug columns per head

FP = mybir.dt.float32
BF = mybir.dt.bfloat16
NP_BF = mybir.dt.np(mybir.dt.bfloat16)
AF = mybir.ActivationFunctionType


def _patch_act_tables():
    """Force Ln and Exp to resolve to the combined
    natural_log_exp_and_others set: with the default tables the chooser
    puts them in different sets and reloads ACT tables between every
    ln/exp pair (1.3us each, 16 loads per kernel).  Emptying the two
    narrower sets (order and ids preserved) leaves the combined set as
    the only candidate, so the whole kernel runs on ONE table load."""
    import concourse.hw_specs as hw_specs

    if getattr(bacc, "_act_tables_patched", False):
        return
    orig = hw_specs.get_activation_tables

    def patched(arch):
        t = dict(orig(arch))
        for k in ("exp_and_others", "natural_log"):
            if k in t:
                t[k] = set()
        return t

    bacc.get_activation_tables = patched
    bacc._act_tables_patched = True


def _build_nc():
    _patch_act_tables()
    nc = bacc.Bacc("TRN2", target_bir_lowering=False, debug=False)

    sinT = nc.dram_tensor("sinT", [B, KT, 128, S], BF,
                          kind="ExternalInput")
    wxl = nc.dram_tensor("wxl", [128, KT * FL], BF, kind="ExternalInput")
    bxl = nc.dram_tensor("bxl", [FL, 1], FP, kind="ExternalInput")
    wpl = nc.dram_tensor("wpl", [FL, D], BF, kind="ExternalInput")
    outp = nc.dram_tensor("outp", [B, S, D], BF, kind="ExternalOutput")

    with tile.TileContext(nc) as tc:
        with (
            tc.tile_pool(name="const", bufs=1) as constp,
            tc.tile_pool(name="wpool", bufs=1) as wpool,
            tc.tile_pool(name="sin", bufs=8) as sinp,
            tc.tile_pool(name="pa", bufs=1) as pa,
            tc.tile_pool(name="pb", bufs=2) as pb,
            tc.tile_pool(name="epool", bufs=4) as epool,
            tc.tile_pool(name="tailp", bufs=2) as tailp,
            tc.tile_pool(name="opool", bufs=3) as opool,
            # 8 PSUM banks:
            #   ps_c  2 x [128,1024] = 4 banks (paired score blocks)
            #   ps_y  2 x [65,512]   = 2 banks (per-head Y accumulators)
            #   ps_a  1 x [128,512]  = 1 bank  (proj accum / psn1 / rnb / outproj)
            #   ps_s  1 x [128,512]  = 1 bank  (psn0 / rnb / transposes / outproj)
            tc.tile_pool(name="ps_c", bufs=2, space="PSUM") as ps_c,
            tc.tile_pool(name="ps_y", bufs=1, space="PSUM") as ps_y,
            tc.tile_pool(name="ps_a", bufs=1, space="PSUM") as ps_a,
            tc.tile_pool(name="ps_s", bufs=1, space="PSUM") as ps_s,
        ):
            # ---- constants / weights ----
            ident = constp.tile([128, 128], FP, tag="ident")
            masks.make_identity(nc, ident[:])

            # ones2[:, h] = 1 on partitions h*64..h*64+63 (norm reductions)
            ones2_f = constp.tile([128, 2], FP, tag="ones2f")
            nc.vector.memset(ones2_f[:], 0.0)
            nc.vector.memset(ones2_f[0:64, 0:1], 1.0)
            nc.vector.memset(ones2_f[64:128, 1:2], 1.0)
            ones2 = constp.tile([128, 2], BF, tag="ones2")
            nc.vector.tensor_copy(ones2[:], ones2_f[:])

            # all-ones [128,64] bf16: K=1 broadcast-matmul stationary
            onesb_f = constp.tile([128, 64], FP, tag="onesbf")
            nc.vector.memset(onesb_f[:], 1.0)
            onesb = constp.tile([128, 64], BF, tag="onesb")
            nc.vector.tensor_copy(onesb[:], onesb_f[:])

            ones16_f = constp.tile([128, NT0], FP, tag="ones16f")
            nc.vector.memset(ones16_f[:], 1.0)

            wx_t = wpool.tile([128, KT * FL], BF, tag="wx")
            nc.sync.dma_start(wx_t[:], wxl.ap())
            bx_t = wpool.tile([FL, 1], FP, tag="bx")
            nc.sync.dma_start(bx_t[:], bxl.ap())
            wp_t = wpool.tile([FL, D], BF, tag="wp")
            nc.sync.dma_start(wp_t[:], wpl.ap())

            state = {}

            def a_parts(b):
                """Emitters for phase A of batch b: projection, norms,
                normalization, aug build.  Returned as a list of parts to be
                interleaved into the previous batch's attention chunks."""
                st = {}
                state[b] = st
                sints = {}

                def mk():
                    st["hT"] = pa.tile([128, S], FP, tag="hT", name=f"hT_{b}")
                    st["sqt"] = pa.tile([128, S], BF, tag="sq", name=f"sq_{b}")
                    st["hTn"] = pb.tile([128, S], BF, tag="hTn",
                                        name=f"hTn_{b}")
                    st["aug"] = pb.tile([128, HPC * AUGW], BF, tag="aug",
                                        name=f"aug_{b}")
                    st["outT"] = pb.tile([128, S], BF, tag="outT",
                                         name=f"outT_{b}")
                    st["ysb"] = [
                        pb.tile([F + 1, S], FP, tag=f"ysb{h}",
                                name=f"ysb{h}_{b}")
                        for h in range(HPC)
                    ]
                    st["rn"] = [
                        pa.tile([128, SCH], BF, tag=f"rn{h}",
                                name=f"rn{h}_{b}")
                        for h in range(HPC)
                    ]

                def a1d():
                    # one 512KB DMA per k-tile (sync-engine dispatch is
                    # ~650ns per DMA; fewer, bigger transfers).  Issued a
                    # full batch ahead so the proj matmuls never reach the
                    # in-order PE queue head while their data is in flight
                    # (head-of-line blocking there starves ACT).
                    for k in range(KT):
                        sint = sinp.tile([128, S], BF, tag="sin",
                                         name=f"sin_{b}_{k}")
                        nc.sync.dma_start(sint[:], sinT.ap()[b, k])
                        sints[k] = sint

                def a1q(half, q):
                    # one half (4 k-tiles) of a 512-wide projection accum
                    if half == 0 and q == 0:
                        mk()
                    cs = slice(half * SCH, (half + 1) * SCH)
                    if q == 0:
                        st["pshT"] = ps_a.tile([128, SCH], FP, tag="a",
                                               name=f"pshT_{b}_{half}")
                    pshT = st["pshT"]
                    for k in range(q * KT // 2, (q + 1) * KT // 2):
                        nc.tensor.matmul(
                            pshT[:], wx_t[:, k * FL:(k + 1) * FL],
                            sints[k][:, cs],
                            start=(k == 0), stop=(k == KT - 1),
                        )
                    if q == 1:
                        nc.vector.tensor_scalar_add(st["hT"][:, cs], pshT[:],
                                                    bx_t[:])
                        nc.vector.tensor_mul(st["sqt"][:, cs], st["hT"][:, cs],
                                             st["hT"][:, cs])

                def norms(h):
                    # norm^2 rows for head h -> psum partitions {0,32,64,96}
                    # (chunk c at row c*32), then rnorm = exp(-0.5*ln(x)):
                    # same ACT table set as the attention exp, no table swaps.
                    # Compact: the psum slot is created and consumed within
                    # this one part (the 1-buf pools rotate with outproj).
                    pool = ps_s if h == 0 else ps_a
                    psn = pool.tile([128, SCH], FP, tag="s" if h == 0
                                    else "a", name=f"psn{h}_{b}")
                    for c in range(NCH):
                        cs = slice(c * SCH, (c + 1) * SCH)
                        nc.tensor.matmul(psn[c * 32:c * 32 + 1, :],
                                         ones2[:, h:h + 1], st["sqt"][:, cs],
                                         start=True, stop=True,
                                         tile_position=(0, c * 32))
                    lnt = pa.tile([128, SCH], FP, tag="lnt",
                                  name=f"lnt_{b}_{h}")
                    nc.scalar.activation(lnt[:], psn[:], AF.Ln)
                    nc.scalar.activation(st["rn"][h][:], lnt[:], AF.Exp,
                                         scale=-0.5)

                def rnb(c):
                    # broadcast rnorm rows to [128,512] via K=1 matmuls,
                    # then one mul makes the normalized hTn chunk
                    pool = ps_s if c % 2 == 0 else ps_a
                    cs = slice(c * SCH, (c + 1) * SCH)
                    prn = pool.tile([128, SCH], FP, tag="s" if c % 2 == 0
                                    else "a", name=f"prn_{b}_{c}")
                    r = c * 32
                    nc.tensor.matmul(prn[0:64, :], onesb[r:r + 1, :],
                                     st["rn"][0][r:r + 1, :],
                                     start=True, stop=True,
                                     tile_position=(r, 0))
                    nc.tensor.matmul(prn[64:128, :], onesb[r:r + 1, :],
                                     st["rn"][1][r:r + 1, :],
                                     start=True, stop=True,
                                     tile_position=(r, 64))
                    nc.vector.tensor_mul(st["hTn"][:, cs], st["hT"][:, cs],
                                         prn[:])

                def a3(q):
                    # 4 transposes: hT [f,t] -> aug [t,f] for both heads
                    for t0 in range(q * 4, q * 4 + 4):
                        pool = ps_s if t0 % 2 == 0 else ps_a
                        pst = pool.tile([128, 128], FP, tag="s" if t0 % 2 == 0
                                        else "a", name=f"pst_{b}_{t0}")
                        nc.tensor.transpose(
                            pst[:], st["hT"][:, t0 * 128:(t0 + 1) * 128],
                            ident[:]
                        )
                        dst = st["aug"][:].rearrange(
                            "p (h t f) -> p h t f", h=HPC, f=F + 1
                        )[:, :, t0, 0:F]
                        src = pst[:].rearrange("p (h f) -> p h f", h=HPC)
                        nc.vector.tensor_copy(dst, src)

                def a4():
                    for h in range(HPC):
                        ones_col = st["aug"][:].rearrange(
                            "p (h t f) -> p h t f", h=HPC, f=F + 1
                        )[:, h, :, F:F + 1]
                        nc.vector.tensor_copy(ones_col, ones16_f[:])

                parts = (
                    [lambda: a1q(0, 0), lambda: a1q(0, 1),
                     lambda: a1q(1, 0), lambda: a1q(1, 1),
                     lambda: a1q(2, 0), lambda: a1q(2, 1),
                     lambda: a1q(3, 0), lambda: a1q(3, 1),
                     lambda: norms(0), lambda: norms(1)]
                    + [lambda c=c: rnb(c) for c in range(NCH)]
                    + [lambda q=q: a3(q) for q in range(4)]
                    + [a4]
                )
                return a1d, parts

            def b_chunk(b, c, fillers=(), end_fillers=()):
                """One 512-wide s-chunk: both heads, all 16 t-blocks."""
                st = state[b]
                cs = slice(c * SCH, (c + 1) * SCH)
                fillers = list(fillers)
                psy = [
                    ps_y.tile([F + 1, SCH], FP, tag=f"y{h}",
                              name=f"psy{h}_{b}_{c}")
                    for h in range(HPC)
                ]
                aug4 = st["aug"][:].rearrange("p (h t f) -> p h t f",
                                              h=HPC, f=F + 1)
                for t0 in range(NT0):
                    ts0 = slice(t0 * 128, (t0 + 1) * 128)
                    psc = ps_c.tile([128, 2 * SCH], FP, tag="c",
                                    name=f"psc_{b}_{c}_{t0}")
                    for h in range(HPC):
                        hs = slice(h * F, (h + 1) * F)
                        nc.tensor.matmul(
                            psc[:, h * SCH:(h + 1) * SCH],
                            st["hTn"][hs, ts0], st["hTn"][hs, cs],
                            start=True, stop=True,
                        )
                    et = epool.tile([128, 2 * SCH], BF, tag="E",
                                    name=f"E_{b}_{c}_{t0}")
                    nc.scalar.activation(et[:], psc[:], AF.Exp)
                    for h in range(HPC):
                        nc.tensor.matmul(
                            psy[h][:], aug4[:, h, t0, :],
                            et[:, h * SCH:(h + 1) * SCH],
                            start=(t0 == 0), stop=(t0 == NT0 - 1),
                        )
                    if fillers:
                        fillers.pop(0)()
                # release the psy banks quickly into the per-head accums
                for h in range(HPC):
                    nc.vector.tensor_copy(st["ysb"][h][:, cs], psy[h][:])
                for part in fillers:
                    part()
                for part in end_fillers:
                    part()

            def tail(b, h, c):
                """Divide one 512-chunk of accumulated Y by the softmax
                denominators (quarter granularity keeps each DVE chain
                short so chunk boundaries never wait on a long tail)."""
                st = state[b]
                cs = slice(c * SCH, (c + 1) * SCH)
                rdsrc = tailp.tile([1, SCH], FP, tag="rdsrc",
                                   name=f"rdsrc_{b}_{h}_{c}")
                nc.vector.tensor_copy(rdsrc[:], st["ysb"][h][F:F + 1, cs])
                rd = tailp.tile([1, SCH], FP, tag="rd",
                                name=f"rd_{b}_{h}_{c}")
                nc.vector.reciprocal_approx_fast(rd[:], rdsrc[:])
                rdb = tailp.tile([F, SCH], FP, tag="rdb",
                                 name=f"rdb_{b}_{h}_{c}")
                nc.gpsimd.partition_broadcast(rdb[:], rd[:])
                nc.vector.tensor_mul(st["outT"][h * F:(h + 1) * F, cs],
                                     st["ysb"][h][0:F, cs], rdb[:])

            def c_parts(b):
                st = state[b]

                def c1(sb, scalar_copy=False):
                    ss = slice(sb * 128, (sb + 1) * 128)
                    ot = opool.tile([128, D], BF, tag="osb",
                                    name=f"ot_{b}_{sb}")
                    for n in range(D // 512):
                        pool = ps_a if n == 0 else ps_s
                        psp = pool.tile([128, 512], FP, tag="a" if n == 0
                                        else "s", name=f"psp_{b}_{sb}_{n}")
                        nc.tensor.matmul(
                            psp[:], st["outT"][:, ss],
                            wp_t[:, n * 512:(n + 1) * 512],
                            start=True, stop=True,
                        )
                        if scalar_copy and n == 0:
                            nc.scalar.copy(ot[:, n * 512:(n + 1) * 512],
                                           psp[:])
                        else:
                            nc.vector.tensor_copy(
                                ot[:, n * 512:(n + 1) * 512], psp[:])
                    nc.sync.dma_start(outp.ap()[b, ss, :], ot[:])

                return [
                    lambda sb=sb: c1(sb, scalar_copy=(b == B - 1 and sb >= 12))
                    for sb in range(S // 128)
                ]

            # ---- software-pipelined emission ----
            # Chunks of batch b interleave with phase A of b+1 and the
            # output projections whose outT halves are complete.  Each
            # batch's sin DMAs are issued one full batch early (chunk 3 of
            # b-1 runs a1d of b+1) so proj matmuls never wait on DMA.
            cl = {}
            dma0, parts0 = a_parts(0)
            dma0()
            # warm the PE's HAM clock gate during the sin DMA flight so the
            # projection runs at 2.4GHz instead of the cold 1.2GHz
            warm_src = constp.tile([128, 512], BF, tag="warm")
            nc.vector.memset(warm_src[:], 0.0)
            for w in range(16):
                pswm = ps_s.tile([128, 512], FP, tag="s", name=f"warm_{w}")
                nc.tensor.matmul(pswm[0:64, :], onesb[:, 0:64], warm_src[:],
                                 start=True, stop=True)
            for part in parts0:
                part()
            nxt = a_parts(1) if B > 1 else None
            if nxt:
                nxt[0]()

            def T(b, h, c):
                return lambda: tail(b, h, c)

            for b in range(B):
                cl[b] = c_parts(b)
                last = b == B - 1
                ap = nxt[1] if nxt else []
                nxt2 = a_parts(b + 2) if b + 2 < B else None
                cp = ((cl[b - 1][4:8], cl[b - 1][8:12], cl[b - 1][12:16])
                      if b >= 1 else ([], [], []))
                ccur = cl[b]
                dman = [nxt2[0]] if nxt2 else []
                if not last:
                    plan = [
                        (0, ap[0:5] + cp[0], []),
                        (1, ap[5:10] + cp[1] + [T(b, 0, 0), T(b, 1, 0)], []),
                        (2, ap[10:19] + cp[2], []),
                        (3, [T(b, 0, 1), T(b, 1, 1), T(b, 0, 2), T(b, 1, 2)]
                         + ccur[0:4] + dman,
                         [T(b, 0, 3), T(b, 1, 3)]),
                    ]
                else:
                    plan = [
                        (0, cp[0], []),
                        (1, cp[1] + [T(b, 0, 0), T(b, 1, 0)], []),
                        (2, cp[2] + ccur[0:4] + [T(b, 0, 1), T(b, 1, 1)], []),
                        (3, [T(b, 0, 2), T(b, 1, 2)] + ccur[4:12],
                         [T(b, 0, 3), T(b, 1, 3)]),
                    ]
                for c, fillers, endf in plan:
                    b_chunk(b, c, fillers, endf)
                nxt = nxt2
            for part in cl[B - 1][12:16]:
                part()

    nc.compile()
    return nc

_NC_CACHE = []


def _get_nc():
    if not _NC_CACHE:
        _NC_CACHE.append(_build_nc())
    return _NC_CACHE[0]


def make_in_maps(sin, Wx, bx, Wp):
    """Host-side sharding: per-core input dicts."""
    # [B, D, S] -> contiguous slabs [B, KT, 128, S]: each sin DMA is one
    # contiguous 512KB read
    sinT = np.ascontiguousarray(
        np.transpose(sin, (0, 2, 1)).reshape(B, KT, 128, S).astype(NP_BF)
    )
    in_maps = []
    for c in range(NCORES):
        hs = slice(c * HPC, (c + 1) * HPC)
        # [D, FL] stacked head projections -> [128, KT*FL] k-tile-major
        wxl = np.concatenate([Wx[h] for h in range(c * HPC, (c + 1) * HPC)],
                             axis=1)
        wxl = np.ascontiguousarray(
            wxl.reshape(KT, 128, FL).transpose(1, 0, 2).reshape(128, KT * FL)
        ).astype(NP_BF)
        bxl = np.ascontiguousarray(bx[hs].reshape(FL, 1))
        wpl = np.ascontiguousarray(Wp[c * FL:(c + 1) * FL, :]).astype(NP_BF)
        in_maps.append({"sinT": sinT, "wxl": wxl, "bxl": bxl, "wpl": wpl})
    return in_maps


def make_runner(sin, Wx, bx, Wp):
    """Build a repeat-callable single-execution runner with device-resident
    inputs.

    Outputs are fed back as the donated output buffers, so each call is
    dispatch + device execution only (no host transfers). Returns
    (run_once, block) where run_once() dispatches one execution
    asynchronously and block() waits for all dispatched work.
    """
    import jax
    from concourse import bass2jax as b2j
    from concourse import mybir as _mb

    nc = _get_nc()
    b2j.install_neuronx_cc_hook()
    in_maps = make_in_maps(
        np.asarray(sin, np.float32), np.asarray(Wx, np.float32),
        np.asarray(bx, np.float32), np.asarray(Wp, np.float32),
    )

    in_names, out_names, out_avals, zero_outs = [], [], [], []
    for alloc in nc.m.functions[0].allocations:
        if not isinstance(alloc, _mb.MemoryLocationSet):
            continue
        name = alloc.memorylocations[0].name
        if alloc.kind == "ExternalInput":
            if nc.partition_id_tensor is None or name != nc.partition_id_tensor.name:
                in_names.append(name)
        elif alloc.kind == "ExternalOutput":
            out_names.append(name)
            shape = tuple(alloc.tensor_shape)
            dtype = _mb.dt.np(alloc.dtype)
            out_avals.append(jax.core.ShapedArray(shape, dtype))
            zero_outs.append(np.zeros(shape, dtype))
    n_params = len(in_names)
    n_outs = len(out_avals)
    all_names = in_names + out_names
    donate = tuple(range(n_params, n_params + n_outs))

    pid_name = nc.partition_id_tensor.name if nc.partition_id_tensor else None
    body_names = all_names + ([pid_name] if pid_name else [])

    def _exec_once(ins_, outs_):
        operands = list(ins_) + list(outs_)
        if pid_name:
            operands.append(b2j.partition_id_tensor())
        outs = b2j._bass_exec_p.bind(
            *operands,
            out_avals=tuple(out_avals),
            in_names=tuple(body_names),
            out_names=tuple(out_names),
            lowering_input_output_aliases=(),
            sim_require_finite=True,
            sim_require_nnan=True,
            nc=nc,
        )
        return tuple(outs)

    def _body(*args):
        return _exec_once(args[:n_params], args[n_params:])

    devices = jax.devices()[:NCORES]
    mesh = b2j.Mesh(np.asarray(devices), ("core",))
    in_specs = (b2j.PartitionSpec("core"),) * (n_params + n_outs)
    out_specs = (b2j.PartitionSpec("core"),) * n_outs
    sharded = jax.jit(
        b2j.shard_map(_body, mesh=mesh, in_specs=in_specs,
                      out_specs=out_specs, check_rep=False),
        donate_argnums=donate, keep_unused=True,
    )
    sharding = jax.sharding.NamedSharding(mesh, b2j.PartitionSpec("core"))
    concat_in = [
        jax.device_put(
            np.concatenate([np.asarray(in_maps[c][nm]) for c in range(NCORES)],
                           axis=0),
            sharding,
        )
        for nm in in_names
    ]
    outs = [
        jax.device_put(np.zeros((NCORES * z.shape[0], *z.shape[1:]), z.dtype),
                       sharding)
        for z in zero_outs
    ]
    jax.block_until_ready(concat_in)

    state = {"outs": outs}

    def run_once():
        state["outs"] = sharded(*concat_in, *state["outs"])

    def block():
        jax.block_until_ready(state["outs"])

    return run_once, block


def benchmark(sin, Wx, bx, Wp, iters=10, loop_n=1, runner=None):
    """Timed loop of the compiled executable; returns per-exec ns."""
    import time as _time

    run_once, block = runner or make_runner(sin, Wx, bx, Wp)
    times = []
    for _ in range(iters):
        t0 = _time.perf_counter()
        for _k in range(loop_n):
            run_once()
        block()
        times.append((_time.perf_counter() - t0) * 1e9 / loop_n)
    return times


def kernel(sin, Wx, bx, Wp, bp, _trace=False):
    sin = np.asarray(sin, dtype=np.float32)
    Wx = np.asarray(Wx, dtype=np.float32)
    bx = np.asarray(bx, dtype=np.float32)
    Wp = np.asarray(Wp, dtype=np.float32)
    bp = np.asarray(bp, dtype=np.float32)

    nc = _get_nc()
    in_maps = make_in_maps(sin, Wx, bx, Wp)
    res = run_bass_kernel_spmd(nc, in_maps, list(range(NCORES)), trace=_trace)
    out = np.sum(np.stack([np.asarray(r["outp"], np.float32)
                       for r in res.results]), axis=0) + bp
    if _trace:
        kernel.last_results = res
    return out.astype(np.float32)
